# revision 4
# baseline (speedup 1.0000x reference)
"""Trainium2 Bass kernel for ComputeAngleInput (GNN angular descriptor).

Math (per center c with n=16 neighbors, F=32 features):
  d_jk[j,k]  = |xyz_j[j] - xyz_j[k]|
  tij[j,k]   = dist[c,j];  tik[j,k] = dist[c,k]
  tjk[j,k]   = (d_jk - max(tij,tik) + min(tij,tik)) / (2*min(tij,tik))
  row(j,k)   = [tij, tik, tjk, emb_i(32), emb_j[j]/tij (32), emb_j[k]/tik (32)]
  output     = rows for all j != k (240 rows of 99 floats), plus centers=atom_i_idx

Strategy: data-parallel over centers across 8 cores (1250 each, padded to
1280 = 10 tiles x 128).  Host does the tiny index gathers (atoms_xyz /
embed_table lookups, ~24MB) and packs one 608-float row per center; the
device kernel does the O(C*n*n*(3+3F)) = 950MB materialization, which is
the memory-roofline term.  One center per SBUF partition; the [256,99]
feature block is built with broadcast (stride-0) copies split across
DVE/ACT, and the off-diagonal 240 rows are written to HBM with strided
DMA access patterns (rows 1..255 skipping every 17th = the diagonal).
"""

import numpy as np

import concourse.bass as bass
import concourse.bacc as bacc
import concourse.mybir as mybir
from concourse import tile
from concourse.bass_utils import run_bass_kernel_spmd

F32 = mybir.dt.float32

C_TOT = 10000
N = 16
F = 32
NCORES = 8
P = 128
TILES_PER_CORE = 10
C_CORE = P * TILES_PER_CORE          # 1280 padded centers per core
C_PAD = C_CORE * NCORES              # 10240
ROW_IN = 16 + 3 * 16 + 32 + N * F    # 608 floats per packed input row
NROW = N * (N - 1)                   # 240 output rows per center
NFEAT = 3 + 3 * F                    # 99
ROW_OUT = NROW * NFEAT               # 23760 floats per center

_CACHED_NC = None


def _build_nc():
    nc = bacc.Bacc("TRN2", target_bir_lowering=False, debug=False)
    inp = nc.declare_dram_parameter("packed", [C_CORE, ROW_IN], F32, isOutput=False)
    out = nc.declare_dram_parameter("out", [C_CORE, ROW_OUT], F32, isOutput=True)

    TT = mybir.AluOpType
    ACTF = mybir.ActivationFunctionType

    with tile.TileContext(nc) as tc:
        with (
            tc.tile_pool(name="io", bufs=3) as io_pool,
            tc.tile_pool(name="tmp", bufs=2) as tmp_pool,
            tc.tile_pool(name="ang", bufs=3) as ang_pool,
        ):
            for t in range(TILES_PER_CORE):
                c0 = t * P
                pk = io_pool.tile([P, ROW_IN], F32, tag="pk")
                nc.sync.dma_start(out=pk[:, :], in_=inp[c0 : c0 + P, :])

                dist = pk[:, 0:16]
                xs = pk[:, 16:32]
                ys = pk[:, 32:48]
                zs = pk[:, 48:64]
                embi = pk[:, 64:96]
                embj = pk[:, 96:608]

                # 1 / dist  (dist in [0.5, 4.5], no zero risk)
                invd = tmp_pool.tile([P, 16], F32, tag="invd")
                nc.vector.reciprocal(invd[:, :], dist)

                # emb_j[j,f] / dist[j]
                embjs = tmp_pool.tile([P, 512], F32, tag="embjs")
                nc.vector.tensor_tensor(
                    out=embjs[:, :].rearrange("p (j f) -> p j f", j=N),
                    in0=embj.rearrange("p (j f) -> p j f", j=N),
                    in1=invd[:, :].unsqueeze(2).broadcast_to([P, N, F]),
                    op=TT.mult,
                )

                # pairwise squared distance among the 16 neighbors
                a = tmp_pool.tile([P, 256], F32, tag="ta")
                b = tmp_pool.tile([P, 256], F32, tag="tb")
                dsq = tmp_pool.tile([P, 256], F32, tag="dsq")
                av = a[:, :].rearrange("p (j k) -> p j k", j=N)
                bv = b[:, :].rearrange("p (j k) -> p j k", j=N)
                dsqv = dsq[:, :].rearrange("p (j k) -> p j k", j=N)
                for i, w in enumerate((xs, ys, zs)):
                    wj = w.unsqueeze(2).broadcast_to([P, N, N])
                    wk = w.unsqueeze(1).broadcast_to([P, N, N])
                    nc.vector.tensor_tensor(out=av, in0=wj, in1=wk, op=TT.subtract)
                    if i == 0:
                        nc.vector.tensor_tensor(out=dsqv, in0=av, in1=av, op=TT.mult)
                    else:
                        nc.vector.tensor_tensor(out=bv, in0=av, in1=av, op=TT.mult)
                        nc.vector.tensor_tensor(out=dsqv, in0=dsqv, in1=bv, op=TT.add)

                djk = tmp_pool.tile([P, 256], F32, tag="djk")
                nc.scalar.sqrt(djk[:, :], dsq[:, :])

                dij_b = dist.unsqueeze(2).broadcast_to([P, N, N])
                dik_b = dist.unsqueeze(1).broadcast_to([P, N, N])
                maxd = tmp_pool.tile([P, 256], F32, tag="maxd")
                mind = tmp_pool.tile([P, 256], F32, tag="mind")
                nc.vector.tensor_tensor(
                    out=maxd[:, :].rearrange("p (j k) -> p j k", j=N),
                    in0=dij_b, in1=dik_b, op=TT.max,
                )
                nc.vector.tensor_tensor(
                    out=mind[:, :].rearrange("p (j k) -> p j k", j=N),
                    in0=dij_b, in1=dik_b, op=TT.min,
                )
                rmin = tmp_pool.tile([P, 256], F32, tag="rmin")
                nc.vector.reciprocal(rmin[:, :], mind[:, :])
                # (djk - maxd) * rmin * 0.5 + 0.5  ==  (djk - maxd + mind)/(2*mind)
                nc.vector.tensor_tensor(out=b[:, :], in0=djk[:, :], in1=maxd[:, :], op=TT.subtract)
                nc.vector.tensor_tensor(out=a[:, :], in0=b[:, :], in1=rmin[:, :], op=TT.mult)
                tjk = tmp_pool.tile([P, 256], F32, tag="tjk")
                nc.scalar.activation(tjk[:, :], a[:, :], ACTF.Copy, bias=0.5, scale=0.5)

                embjs_v = embjs[:, :].rearrange("p (j f) -> p j f", j=N)

                # materialize the two half-blocks (j in [0,8) then [8,16))
                for jg in range(2):
                    j0 = 8 * jg
                    ang = ang_pool.tile([P, 128 * NFEAT], F32, tag="ang")
                    angr = ang[:, :].rearrange("p (r c) -> p r c", c=NFEAT)   # r=128
                    angv = ang[:, :].rearrange("p (j k c) -> p j k c", j=8, c=NFEAT)

                    # col 0: tij = dist[j],  col 1: tik = dist[k],  col 2: tjk
                    nc.vector.tensor_copy(
                        out=angv[:, :, :, 0],
                        in_=dist[:, j0 : j0 + 8].unsqueeze(2).broadcast_to([P, 8, N]),
                    )
                    nc.vector.tensor_copy(
                        out=angv[:, :, :, 1],
                        in_=dist.unsqueeze(1).broadcast_to([P, 8, N]),
                    )
                    nc.vector.tensor_copy(
                        out=angv[:, :, :, 2],
                        in_=tjk[:, 128 * jg : 128 * jg + 128].rearrange(
                            "p (j k) -> p j k", j=8
                        ),
                    )
                    # e_i broadcast on the scalar engine (ACT)
                    nc.scalar.copy(
                        out=angr[:, :, 3 : 3 + F],
                        in_=embi.unsqueeze(1).broadcast_to([P, 128, F]),
                    )
                    # e_j = embjs[j] broadcast over k; e_k = embjs[k] broadcast over j
                    nc.vector.tensor_copy(
                        out=angv[:, :, :, 3 + F : 3 + 2 * F],
                        in_=embjs_v[:, j0 : j0 + 8, :]
                        .unsqueeze(2)
                        .broadcast_to([P, 8, N, F]),
                    )
                    nc.vector.tensor_copy(
                        out=angv[:, :, :, 3 + 2 * F : 3 + 3 * F],
                        in_=embjs_v.unsqueeze(1).broadcast_to([P, 8, N, F]),
                    )

                    # Off-diagonal DMA to HBM.  Local flat rows r = (j-j0)*16+k,
                    # 0..127; diagonals sit at r = 17*j - 128*jg.  The off-diag
                    # rows form: jg=0: [1..16],[18..33]x7 groups, then [120..127];
                    # jg=1: [0..7], then [9..24]..x7 groups.  DRAM rows are
                    # contiguous per chunk: jg=0 -> out rows 0..119, jg=1 -> 120..239.
                    orow0 = 120 * jg * NFEAT
                    if jg == 0:
                        g_src = (
                            ang[:, NFEAT : NFEAT + 7 * 17 * NFEAT]
                            .rearrange("p (g x) -> p g x", g=7)[:, :, 0 : 16 * NFEAT]
                        )
                        g_dst = out[c0 : c0 + P, orow0 : orow0 + 112 * NFEAT].rearrange(
                            "c (g x) -> c g x", g=7
                        )
                        t_src = ang[:, 120 * NFEAT : 128 * NFEAT]
                        t_dst = out[
                            c0 : c0 + P, orow0 + 112 * NFEAT : orow0 + 120 * NFEAT
                        ]
                    else:
                        t_src = ang[:, 0 : 8 * NFEAT]
                        t_dst = out[c0 : c0 + P, orow0 : orow0 + 8 * NFEAT]
                        g_src = (
                            ang[:, 9 * NFEAT : 9 * NFEAT + 7 * 17 * NFEAT]
                            .rearrange("p (g x) -> p g x", g=7)[:, :, 0 : 16 * NFEAT]
                        )
                        g_dst = out[
                            c0 : c0 + P, orow0 + 8 * NFEAT : orow0 + 120 * NFEAT
                        ].rearrange("c (g x) -> c g x", g=7)
                    nc.sync.dma_start(out=g_dst, in_=g_src)
                    nc.sync.dma_start(out=t_dst, in_=t_src)
    nc.compile()
    return nc


def _get_nc():
    global _CACHED_NC
    if _CACHED_NC is None:
        _CACHED_NC = _build_nc()
    return _CACHED_NC


def _pack_inputs(atoms_xyz, embed_table, dist_ij, atom_types, atom_i_idx, atom_j_idx):
    atoms_xyz = np.asarray(atoms_xyz, dtype=np.float32)
    embed_table = np.asarray(embed_table, dtype=np.float32)
    dist_ij = np.asarray(dist_ij, dtype=np.float32)
    atom_types = np.asarray(atom_types)
    atom_i_idx = np.asarray(atom_i_idx)
    atom_j_idx = np.asarray(atom_j_idx)

    C = dist_ij.shape[0]
    packed = np.zeros((C_PAD, ROW_IN), dtype=np.float32)
    packed[:, 0:16] = 1.0  # pad rows: dist=1 avoids reciprocal blowups
    emb = embed_table[atom_types]                     # [nAtoms, F]
    xyz_j = atoms_xyz[atom_j_idx]                     # [C, N, 3]
    packed[:C, 0:16] = dist_ij
    packed[:C, 16:64] = np.ascontiguousarray(
        xyz_j.transpose(0, 2, 1)
    ).reshape(C, 48)                                  # x(16) y(16) z(16)
    packed[:C, 64:96] = emb[atom_i_idx]               # emb_i
    packed[:C, 96:608] = emb[atom_j_idx].reshape(C, N * F)
    return packed


def kernel(atoms_xyz, embed_table, dist_ij, atom_types, atom_i_idx, atom_j_idx):
    packed = _pack_inputs(
        atoms_xyz, embed_table, dist_ij, atom_types, atom_i_idx, atom_j_idx
    )
    nc = _get_nc()
    in_maps = [
        {"packed": packed[i * C_CORE : (i + 1) * C_CORE]} for i in range(NCORES)
    ]
    res = run_bass_kernel_spmd(nc, in_maps, list(range(NCORES)))
    outs = [res.results[i]["out"] for i in range(NCORES)]
    full = np.concatenate(outs, axis=0).reshape(C_PAD, NROW, NFEAT)[:C_TOT]
    centers = np.asarray(atom_i_idx).reshape(-1)
    return full, centers
